# revision 1
# baseline (speedup 1.0000x reference)
"""Trainium2 Bass kernel for block-local (sparse) attention.

Problem: B=4, T=4096, C=1024, H=16, hd=64, BLOCK_SIZE=256.
  qkv = x @ Wqkv + bqkv ; block-diagonal attention per (batch, head, block)
  out = attn_out @ Wout + bout

Strategy (8 NeuronCores, data parallel over the 64 token blocks):
  - Core i handles 8 consecutive 256-token blocks (2048 tokens), processed as
    4 chunks of 512 tokens so the projection matmuls run at N=512 (f32r
    matmuls only hide their internal weight load at moving-dim >= ~512).
  - Everything on-chip is transposed (feature-on-partition): the host feeds
    x^T and takes y^T back, so no on-device transposes exist at all.
  - All matmuls run in float32r (full PE rate, ~1.5e-4 rel err).
  - Scores are computed as scoresT[j,i] (key-index on partitions); exp is
    taken without max subtraction (scores ~N(0, 0.17), safe); the softmax
    denominator is folded into the o-matmul as a trailing ones-column of the
    v operand (row 64 of the o psum = denominator), then: DVE cross-quadrant
    copy down -> reciprocal -> gpsimd partition-broadcast -> one DVE multiply
    (odd heads write cross-quadrant into lanes 64:127 of the K-tile).
  - Weight layouts are pre-packed on the host so every DMA is wide and
    contiguous; q-scale (hd^-0.5) folds into Wq; the v-bias folds into the
    output bias (softmax rows sum to 1). wqk streams per chunk (SBUF budget);
    wv/wout stay resident.
"""
import numpy as np

import concourse.bass as bass
import concourse.mybir as mybir
import concourse.tile as tile
from concourse import bacc

P = 128
B, T, C = 4, 4096, 1024
H = 16
HD = 64
BS = 256                    # attention block size
NB_TOTAL = (B * T) // BS    # 64 blocks total
N_CORES = 8
NB = NB_TOTAL // N_CORES    # 8 blocks per core
TOK = NB * BS               # 2048 tokens per core
KT = C // P                 # 8 contraction tiles
NPAIR = H // 2              # 8 head pairs
TCH = 512                   # projection chunk (2 blocks)
NCH = TOK // TCH            # 4 chunks per core

f32 = mybir.dt.float32
f32r = mybir.dt.float32r
bf16 = mybir.dt.bfloat16
ATT_DT = f32r   # attention operand dtype: f32r (accurate) or bf16 (fast)


def _build(reps: int = 1, variant: str = 'full'):
    nc = bacc.Bacc(None)

    # x^T pre-tiled: [128, KT, NCH, TCH]
    xT = nc.dram_tensor("xT", [P, KT * NCH * TCH], f32r, kind="ExternalInput")
    # wqk packed m-major for streaming: free = (m*KT + k)*128 + j
    wqk = nc.dram_tensor("wqk", [P, 16 * KT * P], f32r, kind="ExternalInput")
    # wv packed: free = k*1024 + (64h + d)
    wv = nc.dram_tensor("wv", [P, KT * C], f32r, kind="ExternalInput")
    # wout packed: free = (k*8 + t)*128 + e
    wout = nc.dram_tensor("wout", [P, KT * 8 * P], f32r, kind="ExternalInput")
    bqk = nc.dram_tensor("bqk", [P, 16], f32, kind="ExternalInput")
    bout = nc.dram_tensor("bout", [P, 8], f32, kind="ExternalInput")
    # y^T: free = (t_etile*NCH + c)*TCH + i
    yT = nc.dram_tensor("yT", [P, 8 * NCH * TCH], f32, kind="ExternalOutput")

    with tile.TileContext(nc) as tc:
        with (
            tc.tile_pool(name="wpool", bufs=1) as wpool,
            tc.tile_pool(name="wqkpool", bufs=5) as wqkpool,
            tc.tile_pool(name="xpool", bufs=2) as xpool,
            tc.tile_pool(name="qkpool", bufs=16) as qkpool,
            tc.tile_pool(name="vpool", bufs=5) as vpool,
            tc.tile_pool(name="epool", bufs=5) as epool,
            tc.tile_pool(name="rpool", bufs=4) as rpool,
            tc.tile_pool(name="opool", bufs=8) as opool,
            tc.tile_pool(name="ypool", bufs=2) as ypool,
            tc.tile_pool(name="pbig", bufs=2, space="PSUM") as pbig,
            tc.tile_pool(name="psc", bufs=4, space="PSUM") as psc,
            tc.tile_pool(name="ppo", bufs=2, space="PSUM") as ppo,
        ):
            xT_r = xT[:].rearrange("p (k c n) -> p k c n", k=KT, c=NCH)
            wqk_r = wqk[:].rearrange("p (m rest) -> p m rest", m=16)

            # --- prologue: chunk-0 x first, then small consts, then weights ---
            if reps == 1:
                xt0 = xpool.tile([P, KT * TCH], f32r, tag="x")
                nc.sync.dma_start(out=xt0[:].rearrange("p (k n) -> p k n", k=KT),
                                  in_=xT_r[:, :, 0, :])
            bqk_t = wpool.tile([P, 16], f32)
            nc.sync.dma_start(out=bqk_t[:], in_=bqk[:])
            bout_t = wpool.tile([P, 8], f32)
            nc.sync.dma_start(out=bout_t[:], in_=bout[:])
            ones_f = wpool.tile([P, 16], f32)
            nc.vector.memset(ones_f[:], 1.0)
            ones16 = wpool.tile([P, 16], ATT_DT)
            nc.vector.tensor_copy(ones16[:], ones_f[:])

            # chunk-0 wqk prefetch BEFORE the big resident weight DMAs
            # (only for reps==1; cross-loop tile reuse deadlocks under For_i)
            wqk0 = []
            if reps == 1:
                for m in range(16):
                    wm = wqkpool.tile([P, KT * P], f32r, tag="wqk", name=f"wqk0_{m}")
                    nc.sync.dma_start(out=wm[:], in_=wqk_r[:, m, :])
                    wqk0.append(wm)

            wv_t = wpool.tile([P, KT * C], f32r)
            for k in range(KT):
                nc.sync.dma_start(out=wv_t[:, k * C:(k + 1) * C],
                                  in_=wv[:, k * C:(k + 1) * C])
            wout_t = wpool.tile([P, KT * 8 * P], f32r)
            for k in range(KT):
                nc.sync.dma_start(out=wout_t[:, k * 8 * P:(k + 1) * 8 * P],
                                  in_=wout[:, k * 8 * P:(k + 1) * 8 * P])

            def chunk_body(c):
                # 0. x^T chunk [128, KT*512]
                if c == 0 and reps == 1:
                    xt = xt0
                else:
                    xt = xpool.tile([P, KT * TCH], f32r, tag="x")
                    nc.sync.dma_start(
                        out=xt[:].rearrange("p (k n) -> p k n", k=KT),
                        in_=xT_r[:, :, c, :])
                # 1. qk projection: 16 m-tiles, N=512; wqk streamed per m-tile
                qk = []
                for m in range(16):
                    if c == 0 and reps == 1:
                        wm = wqk0[m]
                    else:
                        wm = wqkpool.tile([P, KT * P], f32r, tag="wqk")
                        nc.sync.dma_start(out=wm[:], in_=wqk_r[:, m, :])
                    pt = pbig.tile([P, TCH], f32, tag="big")
                    for k in range(KT):
                        nc.tensor.matmul(
                            pt[:], wm[:, k * P:(k + 1) * P],
                            xt[:, k * TCH:(k + 1) * TCH],
                            start=(k == 0), stop=(k == KT - 1))
                    st = qkpool.tile([P, TCH], ATT_DT, tag="qk")
                    nc.scalar.activation(st[:], pt[:],
                                         mybir.ActivationFunctionType.Identity,
                                         bias=bqk_t[:, m:m + 1])
                    qk.append(st)
                # 2. v projection into v65 tiles [128, 16*65] (ones col per head)
                vt = []
                for ts in range(4):
                    v_sb = vpool.tile([P, 16 * 65], ATT_DT, tag="v")
                    for dch in range(2):
                        pt = pbig.tile([P, 512], f32, tag="big")
                        for k in range(KT):
                            nc.tensor.matmul(
                                pt[:],
                                xt[:, k * TCH + ts * P: k * TCH + (ts + 1) * P],
                                wv_t[:, k * C + dch * 512: k * C + (dch + 1) * 512],
                                start=(k == 0), stop=(k == KT - 1))
                        nc.vector.tensor_copy(
                            v_sb[:, dch * 8 * 65:(dch + 1) * 8 * 65]
                            .rearrange("p (h cc) -> p h cc", h=8)[:, :, 0:HD],
                            pt[:].rearrange("p (h cc) -> p h cc", h=8))
                    nc.vector.tensor_copy(
                        v_sb[:].rearrange("p (h cc) -> p h cc", h=16)[:, :, HD:65],
                        ones16[:].unsqueeze(2))
                    vt.append(v_sb)
                # 3. attention: 2 blocks x 8 pairs
                on_tiles = [opool.tile([P, TCH], f32r, tag="on", name=f"on_{c}_{kk}")
                            for kk in range(8)]
                if variant == 'noattn':
                    for kk in range(8):
                        nc.vector.tensor_copy(on_tiles[kk][:], qk[kk][:])
                for bl in range(2 if variant != 'noattn' else 0):
                    co = bl * BS    # chunk-local column offset of this block
                    for p_ in range(NPAIR):
                        qt, kt_ = qk[p_], qk[8 + p_]
                        ex = [None, None]
                        pss = [psc.tile([P, 2 * BS], f32, tag="sc", name=f"sc{hh}")
                               for hh in range(2)]
                        for jt in range(2):
                            for hh in range(2):
                                lo, hi = hh * HD, (hh + 1) * HD
                                nc.tensor.matmul(
                                    pss[hh][:, jt * BS:(jt + 1) * BS],
                                    kt_[lo:hi, co + jt * P: co + (jt + 1) * P],
                                    qt[lo:hi, co:co + BS], start=True, stop=True)
                        for hh in range(2):
                            e = epool.tile([P, 2 * BS], ATT_DT, tag="e")
                            nc.scalar.activation(
                                e[:], pss[hh][:], mybir.ActivationFunctionType.Exp)
                            ex[hh] = e
                        for hh in range(2):
                            h = 2 * p_ + hh
                            po = ppo.tile([65, BS], f32, tag="po")
                            for jt in range(2):
                                nc.tensor.matmul(
                                    po[:],
                                    vt[2 * bl + jt][:, h * 65:(h + 1) * 65],
                                    ex[hh][:, jt * BS:(jt + 1) * BS],
                                    start=(jt == 0), stop=(jt == 1))
                            if variant == 'nonorm':
                                nc.vector.tensor_copy(
                                    on_tiles[p_][hh * HD:(hh + 1) * HD, co:co + BS],
                                    po[0:HD, :])
                            else:
                                rcp = rpool.tile([1, BS], f32, tag="rcp")
                                nc.vector.reciprocal(rcp[:], po[64:65, :])
                                rcr = rpool.tile([P, BS], f32, tag="rcr")
                                nc.gpsimd.partition_broadcast(rcr[:], rcp[:])
                                nc.vector.tensor_mul(
                                    on_tiles[p_][hh * HD:(hh + 1) * HD, co:co + BS],
                                    po[0:HD, :], rcr[0:HD, :])
                # 4. out projection, N=512
                for t in range(8):
                    pt = pbig.tile([P, TCH], f32, tag="big")
                    for kk in range(KT):
                        nc.tensor.matmul(
                            pt[:], wout_t[:, (kk * 8 + t) * P:(kk * 8 + t + 1) * P],
                            on_tiles[kk][:], start=(kk == 0), stop=(kk == KT - 1))
                    yt = ypool.tile([P, TCH], f32, tag="y")
                    nc.scalar.activation(yt[:], pt[:],
                                         mybir.ActivationFunctionType.Identity,
                                         bias=bout_t[:, t:t + 1])
                    nc.sync.dma_start(
                        out=yT[:, (t * NCH + c) * TCH:(t * NCH + c + 1) * TCH],
                        in_=yt[:])

            def all_chunks():
                for c in range(NCH):
                    chunk_body(c)

            if reps == 1:
                all_chunks()
            else:
                with tc.For_i(0, reps, 1):
                    all_chunks()
    nc.finalize()
    return nc


def prep_inputs(x, Wqkv, bqkv, Wout, bout):
    """Host-side shard + repack. Returns list of 8 per-core input dicts."""
    x = np.asarray(x, dtype=np.float32)
    Wqkv = np.asarray(Wqkv, dtype=np.float32)
    bqkv = np.asarray(bqkv, dtype=np.float32)
    Wout = np.asarray(Wout, dtype=np.float32)
    bout = np.asarray(bout, dtype=np.float32)

    scale = 1.0 / np.sqrt(HD)
    W3 = Wqkv.reshape(C, H, 3 * HD)
    b3 = bqkv.reshape(H, 3 * HD)
    Wq = W3[:, :, 0:HD] * scale          # [C, H, 64]
    Wk = W3[:, :, HD:2 * HD]
    Wv = W3[:, :, 2 * HD:3 * HD]
    bq = b3[:, 0:HD] * scale
    bk = b3[:, HD:2 * HD]
    bv = b3[:, 2 * HD:3 * HD]

    # m-tiles: m<8 -> [Wq_{2m} | Wq_{2m+1}], m>=8 -> k-pairs
    mt = np.empty((C, 16, P), dtype=np.float32)
    for m in range(8):
        mt[:, m, 0:HD] = Wq[:, 2 * m]
        mt[:, m, HD:P] = Wq[:, 2 * m + 1]
        mt[:, 8 + m, 0:HD] = Wk[:, 2 * m]
        mt[:, 8 + m, HD:P] = Wk[:, 2 * m + 1]
    # -> [128, m, k, 128] m-major flat
    wqk_h = np.ascontiguousarray(
        mt.reshape(KT, P, 16, P).transpose(1, 2, 0, 3).reshape(P, 16 * KT * P))

    wv_full = Wv.reshape(C, H * HD)
    wv_h = np.ascontiguousarray(
        wv_full.reshape(KT, P, C).transpose(1, 0, 2).reshape(P, KT * C))

    wout_h = np.ascontiguousarray(
        Wout.reshape(KT, P, 8, P).transpose(1, 0, 2, 3).reshape(P, KT * 8 * P))

    bqk_h = np.empty((P, 16), dtype=np.float32)
    for m in range(8):
        bqk_h[0:HD, m] = bq[2 * m]
        bqk_h[HD:P, m] = bq[2 * m + 1]
        bqk_h[0:HD, 8 + m] = bk[2 * m]
        bqk_h[HD:P, 8 + m] = bk[2 * m + 1]

    boutp = bout + bv.reshape(H * HD) @ Wout
    bout_h = np.ascontiguousarray(boutp.reshape(8, P).T)

    xb = x.reshape(NB_TOTAL, BS, C)
    in_maps = []
    for core in range(N_CORES):
        blocks = xb[core * NB:(core + 1) * NB]
        xTc = blocks.reshape(TOK, C).T                  # [C, 2048]
        xTt = (xTc.reshape(KT, P, NCH, TCH)
               .transpose(1, 0, 2, 3).reshape(P, KT * NCH * TCH))
        in_maps.append({
            "xT": np.ascontiguousarray(xTt),
            "wqk": wqk_h, "wv": wv_h, "wout": wout_h,
            "bqk": bqk_h, "bout": bout_h,
        })
    return in_maps


def assemble_output(results):
    """results: list of 8 dicts with 'yT' [128, 8*NCH*TCH] -> full y [B, T, C]."""
    y = np.empty((N_CORES, TOK, C), dtype=np.float32)
    for core, r in enumerate(results):
        yT = r["yT"].reshape(P, 8, NCH, TCH)   # [p, etile, c, i]
        yc = yT.transpose(2, 3, 1, 0).reshape(TOK, C)
        y[core] = yc
    return y.reshape(B, T, C)


_CACHED = {}


def kernel(x, Wqkv, bqkv, Wout, bout):
    from concourse.bass_utils import run_bass_kernel_spmd
    if "nc" not in _CACHED:
        _CACHED["nc"] = _build(reps=1)
    in_maps = prep_inputs(x, Wqkv, bqkv, Wout, bout)
    res = run_bass_kernel_spmd(_CACHED["nc"], in_maps, list(range(N_CORES)))
    return assemble_output(res.results)



# revision 43
# speedup vs baseline: 3.6378x; 3.6378x over previous
"""Trainium2 Bass kernel for block-local (sparse) attention.

Problem: B=4, T=4096, C=1024, H=16, hd=64, BLOCK_SIZE=256.
  qkv = x @ Wqkv + bqkv ; block-diagonal attention per (batch, head, block)
  out = attn_out @ Wout + bout

Strategy (8 NeuronCores, data parallel over the 64 token blocks):
  - Core i handles 8 consecutive 256-token blocks (2048 tokens), processed as
    4 chunks of 512 tokens so the projection matmuls run at N=512.
  - Everything on-chip is transposed (feature-on-partition): the host feeds
    x^T and takes y^T back, so no on-device transposes exist at all.
  - All matmul operands are bf16 (PE full rate; psum accumulates f32).
    Halved DMA/SBUF lets ALL weights stay resident across chunks and reps.
  - Scores are computed as scoresT[j,i] (key-index on partitions); exp is
    taken without max subtraction (scores ~N(0, 0.17), safe); the softmax
    denominator is folded into the o-matmul as a trailing ones-column of the
    v operand (row 64 of the o psum = denominator). Both heads of a pair
    share one [65, 512] po psum tile (hh on columns), so the normalization
    chain (DVE reciprocal -> Pool partition-broadcast -> 2 DVE multiplies)
    runs once per (pair, block).
  - Engine placement: qk/y psum->sbuf copies (+bias) on Act via Identity
    (exp and identity share an act table, so no table reloads); v copies on
    DVE; Pool only does the partition broadcast.
  - Software pipelining: the out-projection of chunk c-1 is interleaved into
    the attention pair-loop of chunk c as PE filler work, so the PE's
    in-order queue never stalls on the exp/normalization chains. The last
    chunk's out-projection runs at the loop tail (overlapping the next rep's
    qk projection under For_i).
  - Weight layouts are pre-packed on the host so every DMA is wide and
    contiguous; q-scale (hd^-0.5) folds into Wq; the v-bias folds into the
    output bias (softmax rows sum to 1).
"""
import numpy as np
import ml_dtypes

import concourse.bass as bass
import concourse.mybir as mybir
import concourse.tile as tile
from concourse import bacc

P = 128
B, T, C = 4, 4096, 1024
H = 16
HD = 64
BS = 256                    # attention block size
NB_TOTAL = (B * T) // BS    # 64 blocks total
N_CORES = 8
NB = NB_TOTAL // N_CORES    # 8 blocks per core
TOK = NB * BS               # 2048 tokens per core
KT = C // P                 # 8 contraction tiles
NPAIR = H // 2              # 8 head pairs
TCH = 512                   # projection chunk (2 blocks)
NCH = TOK // TCH            # 4 chunks per core

f32 = mybir.dt.float32
bf16 = mybir.dt.bfloat16
fp8 = mybir.dt.float8e4
BF = ml_dtypes.bfloat16


def _build(reps: int = 1, unroll: bool = False, psc_bufs: int = 3,
           ppo_bufs: int = 3, o_dtype=bf16, norm: str = "pair",
           fill_split: int = 1, skip_ones: bool = False):
    nc = bacc.Bacc(None)

    # x^T pre-tiled: [128, KT, NCH, TCH]
    xT = nc.dram_tensor("xT", [P, KT * NCH * TCH], bf16, kind="ExternalInput")
    # wqk packed m-major: free = (m*KT + k)*128 + j
    wqk = nc.dram_tensor("wqk", [P, 16 * KT * P], bf16, kind="ExternalInput")
    # wv packed: free = k*1024 + (64h + d)
    wv = nc.dram_tensor("wv", [P, KT * C], bf16, kind="ExternalInput")
    # wout packed: free = (k*8 + t)*128 + e
    wout = nc.dram_tensor("wout", [P, KT * 8 * P], bf16, kind="ExternalInput")
    bqk = nc.dram_tensor("bqk", [P, 16], f32, kind="ExternalInput")
    bout = nc.dram_tensor("bout", [P, 8], f32, kind="ExternalInput")
    # y^T: free = (t_etile*NCH + c)*TCH + i
    yT = nc.dram_tensor("yT", [P, 8 * NCH * TCH], bf16, kind="ExternalOutput")

    from contextlib import ExitStack
    with tile.TileContext(nc) as tc:
        with (
            tc.tile_pool(name="wpool", bufs=1) as wpool,
            tc.tile_pool(name="xpool", bufs=2) as xpool,
            tc.tile_pool(name="qkpool", bufs=20) as qkpool,
            tc.tile_pool(name="vpool", bufs=4) as vpool,
            tc.tile_pool(name="epool", bufs=5) as epool,
            tc.tile_pool(name="rcppool", bufs=3) as rcppool,
            tc.tile_pool(name="rcrpool", bufs=2) as rcrpool,
            tc.tile_pool(name="papool", bufs=2) as papool,
            tc.tile_pool(name="opool", bufs=16) as opool,
            tc.tile_pool(name="ypool", bufs=3) as ypool,
            tc.tile_pool(name="pbig", bufs=2, space="PSUM") as pbig,
            tc.tile_pool(name="psc", bufs=psc_bufs, space="PSUM") as psc,
            tc.tile_pool(name="ppo", bufs=ppo_bufs, space="PSUM") as ppo,
            ExitStack() as _es,
        ):
            prcr = (_es.enter_context(
                tc.tile_pool(name="prcr", bufs=2, space="PSUM"))
                if norm in ("pebc", "pediv") else None)
            xT_r = xT[:].rearrange("p (k c n) -> p k c n", k=KT, c=NCH)

            # --- prologue: small consts, then resident weights ---
            ones_bc = None
            if norm in ("pebc", "pediv"):
                ones_bc = wpool.tile([1, P], bf16)
                nc.vector.memset(ones_bc[:], 1.0)
            bqk_t = wpool.tile([P, 16], f32)
            nc.sync.dma_start(out=bqk_t[:], in_=bqk[:])
            bout_t = wpool.tile([P, 8], f32)
            nc.sync.dma_start(out=bout_t[:], in_=bout[:])

            wqk_t = wpool.tile([P, 16 * KT * P], bf16)
            for s in range(4):
                w = 4 * KT * P
                nc.sync.dma_start(out=wqk_t[:, s * w:(s + 1) * w],
                                  in_=wqk[:, s * w:(s + 1) * w])
            wv_t = wpool.tile([P, KT * C], bf16)
            for s in range(4):
                w = 2 * C
                nc.sync.dma_start(out=wv_t[:, s * w:(s + 1) * w],
                                  in_=wv[:, s * w:(s + 1) * w])
            wout_t = wpool.tile([P, KT * 8 * P], bf16)
            for s in range(4):
                w = 2 * 8 * P
                nc.sync.dma_start(out=wout_t[:, s * w:(s + 1) * w],
                                  in_=wout[:, s * w:(s + 1) * w])

            def emit_outproj(on8, cp, t, half=None):
                # half=None: full tile; half=(pt, 0|1): one contraction half,
                # finishing + y output on half 1
                if half is None or half[1] == 0:
                    pt = pbig.tile([P, TCH], f32, tag="big")
                else:
                    pt = half[0]
                ks = range(KT) if half is None else range(
                    half[1] * (KT // 2), (half[1] + 1) * (KT // 2))
                for kk in ks:
                    nc.tensor.matmul(
                        pt[:], wout_t[:, (kk * 8 + t) * P:(kk * 8 + t + 1) * P],
                        on8[kk][:], start=(kk == 0), stop=(kk == KT - 1))
                if half is not None and half[1] == 0:
                    return pt
                yt = ypool.tile([P, TCH], bf16, tag="y")
                nc.scalar.activation(yt[:], pt[:],
                                     mybir.ActivationFunctionType.Identity,
                                     bias=bout_t[:, t:t + 1])
                nc.sync.dma_start(
                    out=yT[:, (t * NCH + cp) * TCH:(t * NCH + cp + 1) * TCH],
                    in_=yt[:])
                return None

            def chunk_body(c, prev):
                # 0. x^T chunk [128, KT*512]
                xt = xpool.tile([P, KT * TCH], bf16, tag="x")
                nc.sync.dma_start(
                    out=xt[:].rearrange("p (k n) -> p k n", k=KT),
                    in_=xT_r[:, :, c, :])
                # 1. qk projection: 16 m-tiles, N=512; wqk resident
                qk = []
                for m in range(16):
                    pt = pbig.tile([P, TCH], f32, tag="big")
                    for k in range(KT):
                        nc.tensor.matmul(
                            pt[:], wqk_t[:, (m * KT + k) * P:(m * KT + k + 1) * P],
                            xt[:, k * TCH:(k + 1) * TCH],
                            start=(k == 0), stop=(k == KT - 1))
                    st = qkpool.tile([P, TCH], bf16, tag="qk")
                    nc.scalar.activation(st[:], pt[:],
                                         mybir.ActivationFunctionType.Identity,
                                         bias=bqk_t[:, m:m + 1])
                    qk.append(st)
                # 2. v projection into per-block v tiles (jt-major halves).
                #    'vones': each head gets 64 ones columns so the o-matmul
                #    replicates the softmax denominator across 64 psum rows
                #    (matmul cost depends only on the moving dim, so this is
                #    free) - no partition broadcast needed downstream.
                VW = 2 * HD if norm in ("vones", "vdiv") else 65
                VHALF = 16 * HD + HD if norm == "vshared" else 16 * VW
                vt = []
                for bl in range(2):
                    v_sb = vpool.tile([P, 2 * VHALF], o_dtype, tag="v")
                    for jt in range(2):
                        ts = 2 * bl + jt
                        half = v_sb[:, jt * VHALF:(jt + 1) * VHALF]
                        for dch in range(2):
                            pt = pbig.tile([P, 512], f32, tag="big")
                            for k in range(KT):
                                nc.tensor.matmul(
                                    pt[:],
                                    xt[:, k * TCH + ts * P: k * TCH + (ts + 1) * P],
                                    wv_t[:, k * C + dch * 512: k * C + (dch + 1) * 512],
                                    start=(k == 0), stop=(k == KT - 1))
                            if norm == "vshared":
                                # dense layout: plain contiguous copy
                                nc.vector.tensor_copy(
                                    half[:, dch * 512:(dch + 1) * 512], pt[:])
                            else:
                                nc.vector.tensor_copy(
                                    half[:, dch * 8 * VW:(dch + 1) * 8 * VW]
                                    .rearrange("p (h cc) -> p h cc", h=8)[:, :, 0:HD],
                                    pt[:].rearrange("p (h cc) -> p h cc", h=8))
                    if not skip_ones:
                        if norm == "vshared":
                            # one shared 64-wide ones block per jt half
                            nc.vector.memset(
                                v_sb[:].rearrange("p (j cc) -> p j cc", j=2)
                                [:, :, 16 * HD:16 * HD + HD], 1.0)
                        else:
                            nc.vector.memset(
                                v_sb[:].rearrange(
                                    "p (j h cc) -> p j h cc", j=2, h=16)
                                [:, :, :, HD:VW], 1.0)
                    vt.append(v_sb)
                # 3. attention, with outproj(c-1) interleaved as PE filler
                on_tiles = [opool.tile([P, TCH], bf16, tag="on",
                                       name=f"on_{c}_{kk}")
                            for kk in range(8)]
                have_fill = prev["on"] is not None
                fill_units = 8 * fill_split if have_fill else 0
                fi = 0
                half_state = {}

                def emit_fill_unit(u):
                    t, part = divmod(u, fill_split)
                    if fill_split == 1:
                        emit_outproj(prev["on"], prev["c"], t)
                    elif part == 0:
                        half_state[t] = emit_outproj(
                            prev["on"], prev["c"], t, half=(None, 0))
                    else:
                        emit_outproj(prev["on"], prev["c"], t,
                                     half=(half_state.pop(t), 1))

                po_all = None
                if norm == "bulk":
                    po_all = papool.tile([65, 16 * 2 * BS], bf16, tag="pa")

                for i, (bl, p_) in enumerate(
                        [(b_, pp) for b_ in range(2) for pp in range(NPAIR)]):
                    co = bl * BS
                    qt, kt_ = qk[p_], qk[8 + p_]
                    pss = [psc.tile([P, 2 * BS], f32, tag="sc", name=f"sc{hh}")
                           for hh in range(2)]
                    for jt in range(2):
                        for hh in range(2):
                            lo, hi = hh * HD, (hh + 1) * HD
                            nc.tensor.matmul(
                                pss[hh][:, jt * BS:(jt + 1) * BS],
                                kt_[lo:hi, co + jt * P: co + (jt + 1) * P],
                                qt[lo:hi, co:co + BS], start=True, stop=True)
                    ex = []
                    for hh in range(2):
                        e = epool.tile([P, 2 * BS], o_dtype, tag="e")
                        nc.scalar.activation(
                            e[:], pss[hh][:], mybir.ActivationFunctionType.Exp)
                        ex.append(e)
                    po = ppo.tile(
                        [2 * HD if norm in ("vones", "vdiv", "vshared") else 65,
                         2 * BS], f32, tag="po")
                    for hh in range(2):
                        h = 2 * p_ + hh
                        if o_dtype == fp8:
                            # fp8 DoubleRow: both jt key-tiles in one matmul
                            # (dim 1 of each AP is the contraction-pair dim)
                            nc.tensor.matmul(
                                po[:, hh * BS:(hh + 1) * BS],
                                vt[bl][:].rearrange(
                                    "p (j hm) -> p j hm", j=2)
                                [:, :, h * VW:(h + 1) * VW],
                                ex[hh][:].rearrange(
                                    "p (j i) -> p j i", j=2),
                                start=True, stop=True,
                                perf_mode=mybir.MatmulPerfMode.DoubleRow)
                        elif norm == "vshared":
                            for jt in range(2):
                                va = vt[bl][:]
                                lhsT = bass.AP(
                                    va.tensor,
                                    va.offset + jt * VHALF + h * HD,
                                    [list(va.ap[0]),
                                     [16 * HD - h * HD, 2], [1, HD]])
                                nc.tensor.matmul(
                                    po[:, hh * BS:(hh + 1) * BS], lhsT,
                                    ex[hh][:, jt * BS:(jt + 1) * BS],
                                    start=(jt == 0), stop=(jt == 1))
                        else:
                            for jt in range(2):
                                nc.tensor.matmul(
                                    po[:, hh * BS:(hh + 1) * BS],
                                    vt[bl][:].rearrange(
                                        "p (j hm) -> p j hm", j=2)
                                    [:, jt, h * VW:(h + 1) * VW],
                                    ex[hh][:, jt * BS:(jt + 1) * BS],
                                    start=(jt == 0), stop=(jt == 1))
                    if norm in ("vones", "vshared"):
                        # po rows 64:128 hold the denominator (64 replicated
                        # copies from the ones columns): one reciprocal, then
                        # the multiplies read it lane-aligned - no broadcast
                        rcp64 = rcppool.tile([HD, 2 * BS], f32, tag="rcp")
                        nc.vector.reciprocal(rcp64[:], po[HD:2 * HD, :])
                        for hh in range(2):
                            nc.vector.tensor_mul(
                                on_tiles[p_][hh * HD:(hh + 1) * HD, co:co + BS],
                                po[0:HD, hh * BS:(hh + 1) * BS],
                                rcp64[0:HD, hh * BS:(hh + 1) * BS])
                    elif norm == "vdiv":
                        # one DVE divide per head: numerator rows 0:64,
                        # replicated denominator rows 64:128
                        for hh in range(2):
                            nc.vector.tensor_tensor(
                                on_tiles[p_][hh * HD:(hh + 1) * HD, co:co + BS],
                                po[0:HD, hh * BS:(hh + 1) * BS],
                                po[HD:2 * HD, hh * BS:(hh + 1) * BS],
                                mybir.AluOpType.divide)
                    elif norm == "pair":
                        rcp = rcppool.tile([1, 2 * BS], f32, tag="rcp")
                        nc.vector.reciprocal(rcp[:], po[64:65, :])
                        rcr = rcrpool.tile([P, 2 * BS], f32, tag="rcr")
                        nc.gpsimd.partition_broadcast(rcr[:], rcp[:])
                        for hh in range(2):
                            nc.vector.tensor_mul(
                                on_tiles[p_][hh * HD:(hh + 1) * HD, co:co + BS],
                                po[0:HD, hh * BS:(hh + 1) * BS],
                                rcr[0:HD, hh * BS:(hh + 1) * BS])
                    elif norm == "pebc":
                        # broadcast 1/den along partitions via a contraction-1
                        # PE matmul (gpsimd partition_broadcast is slow on HW)
                        rcp = rcppool.tile([1, 2 * BS], bf16, tag="rcp")
                        with nc.allow_low_precision(
                                reason="bf16 recip of softmax denom adds "
                                       "~0.4%, well within tolerance"):
                            nc.vector.reciprocal(rcp[:], po[64:65, :])
                        rcr = prcr.tile([P, 2 * BS], f32, tag="rcr")
                        nc.tensor.matmul(rcr[:], ones_bc[:], rcp[:],
                                         start=True, stop=True)
                        for hh in range(2):
                            nc.vector.tensor_mul(
                                on_tiles[p_][hh * HD:(hh + 1) * HD, co:co + BS],
                                po[0:HD, hh * BS:(hh + 1) * BS],
                                rcr[0:HD, hh * BS:(hh + 1) * BS])
                    elif norm == "pediv":
                        # den -> SBUF (Act), PE-broadcast den, DVE divide
                        den = rcppool.tile([1, 2 * BS], bf16, tag="rcp")
                        nc.scalar.activation(
                            den[:], po[64:65, :],
                            mybir.ActivationFunctionType.Identity)
                        dbc = prcr.tile([P, 2 * BS], f32, tag="rcr")
                        nc.tensor.matmul(dbc[:], ones_bc[:], den[:],
                                         start=True, stop=True)
                        for hh in range(2):
                            nc.vector.tensor_tensor(
                                on_tiles[p_][hh * HD:(hh + 1) * HD, co:co + BS],
                                po[0:HD, hh * BS:(hh + 1) * BS],
                                dbc[0:HD, hh * BS:(hh + 1) * BS],
                                mybir.AluOpType.divide)
                    elif norm == "bulk":
                        # psum -> sbuf staging on DVE (Act queue is busy with
                        # exps); normalization happens in a few big ops after
                        # the pair loop (phase B)
                        nc.vector.tensor_copy(
                            po_all[:, i * 2 * BS:(i + 1) * 2 * BS], po[:])
                    else:  # 'none': timing ablation, skips normalization
                        for hh in range(2):
                            nc.vector.tensor_copy(
                                on_tiles[p_][hh * HD:(hh + 1) * HD, co:co + BS],
                                po[0:HD, hh * BS:(hh + 1) * BS])
                    if fi < fill_units and (fill_split == 2 or i % 2 == 1):
                        emit_fill_unit(fi)
                        fi += 1
                if norm == "bulk":
                    # phase B: 4 groups of 4 pair-blocks
                    GW = 4 * 2 * BS
                    for g in range(4):
                        rcp = rcppool.tile([1, GW], bf16, tag="rcp")
                        with nc.allow_low_precision(
                                reason="softmax denom ~256±small; bf16 recip "
                                       "adds ~0.4% well within tolerance"):
                            nc.vector.reciprocal(
                                rcp[:], po_all[64:65, g * GW:(g + 1) * GW])
                        rcr = rcrpool.tile([P, GW], bf16, tag="rcr")
                        nc.gpsimd.partition_broadcast(rcr[:], rcp[:])
                        for ii in range(4):
                            i = g * 4 + ii
                            bl, p_ = divmod(i, NPAIR)
                            co = bl * BS
                            for hh in range(2):
                                nc.vector.tensor_mul(
                                    on_tiles[p_][hh * HD:(hh + 1) * HD,
                                                 co:co + BS],
                                    po_all[0:HD,
                                           i * 2 * BS + hh * BS:
                                           i * 2 * BS + (hh + 1) * BS],
                                    rcr[0:HD,
                                        ii * 2 * BS + hh * BS:
                                        ii * 2 * BS + (hh + 1) * BS])
                prev["on"] = on_tiles
                prev["c"] = c

            def all_chunks():
                prev = {"on": None, "c": None}
                for c in range(NCH):
                    chunk_body(c, prev)
                # tail: last chunk's out-projection (overlaps next rep's
                # qk projection under For_i)
                for t in range(8):
                    emit_outproj(prev["on"], prev["c"], t)

            if reps == 1:
                all_chunks()
            elif unroll:
                for _ in range(reps):
                    all_chunks()
            else:
                with tc.For_i(0, reps, 1):
                    all_chunks()
    nc.finalize()
    return nc


def prep_inputs(x, Wqkv, bqkv, Wout, bout):
    """Host-side shard + repack. Returns list of 8 per-core input dicts."""
    x = np.asarray(x, dtype=np.float32)
    Wqkv = np.asarray(Wqkv, dtype=np.float32)
    bqkv = np.asarray(bqkv, dtype=np.float32)
    Wout = np.asarray(Wout, dtype=np.float32)
    bout = np.asarray(bout, dtype=np.float32)

    scale = 1.0 / np.sqrt(HD)
    W3 = Wqkv.reshape(C, H, 3 * HD)
    b3 = bqkv.reshape(H, 3 * HD)
    Wq = W3[:, :, 0:HD] * scale          # [C, H, 64]
    Wk = W3[:, :, HD:2 * HD]
    Wv = W3[:, :, 2 * HD:3 * HD]
    bq = b3[:, 0:HD] * scale
    bk = b3[:, HD:2 * HD]
    bv = b3[:, 2 * HD:3 * HD]

    # m-tiles: m<8 -> [Wq_{2m} | Wq_{2m+1}], m>=8 -> k-pairs
    mt = np.empty((C, 16, P), dtype=np.float32)
    for m in range(8):
        mt[:, m, 0:HD] = Wq[:, 2 * m]
        mt[:, m, HD:P] = Wq[:, 2 * m + 1]
        mt[:, 8 + m, 0:HD] = Wk[:, 2 * m]
        mt[:, 8 + m, HD:P] = Wk[:, 2 * m + 1]
    # -> [128, m, k, 128] m-major flat
    wqk_h = np.ascontiguousarray(
        mt.reshape(KT, P, 16, P).transpose(1, 2, 0, 3)
        .reshape(P, 16 * KT * P)).astype(BF)

    wv_full = Wv.reshape(C, H * HD)
    wv_h = np.ascontiguousarray(
        wv_full.reshape(KT, P, C).transpose(1, 0, 2).reshape(P, KT * C)
    ).astype(BF)

    wout_h = np.ascontiguousarray(
        Wout.reshape(KT, P, 8, P).transpose(1, 0, 2, 3).reshape(P, KT * 8 * P)
    ).astype(BF)

    bqk_h = np.empty((P, 16), dtype=np.float32)
    for m in range(8):
        bqk_h[0:HD, m] = bq[2 * m]
        bqk_h[HD:P, m] = bq[2 * m + 1]
        bqk_h[0:HD, 8 + m] = bk[2 * m]
        bqk_h[HD:P, 8 + m] = bk[2 * m + 1]

    boutp = bout + bv.reshape(H * HD) @ Wout
    bout_h = np.ascontiguousarray(boutp.reshape(8, P).T)

    xb = x.reshape(NB_TOTAL, BS, C)
    in_maps = []
    for core in range(N_CORES):
        blocks = xb[core * NB:(core + 1) * NB]
        xTc = blocks.reshape(TOK, C).T                  # [C, 2048]
        xTt = (xTc.reshape(KT, P, NCH, TCH)
               .transpose(1, 0, 2, 3).reshape(P, KT * NCH * TCH))
        in_maps.append({
            "xT": np.ascontiguousarray(xTt).astype(BF),
            "wqk": wqk_h, "wv": wv_h, "wout": wout_h,
            "bqk": bqk_h, "bout": bout_h,
        })
    return in_maps


def assemble_output(results):
    """results: list of 8 dicts with 'yT' [128, 8*NCH*TCH] -> full y [B, T, C]."""
    y = np.empty((N_CORES, TOK, C), dtype=np.float32)
    for core, r in enumerate(results):
        yT = np.asarray(r["yT"])
        if yT.dtype != np.float32:
            yT = yT.astype(np.float32)
        yT = yT.reshape(P, 8, NCH, TCH)   # [p, etile, c, i]
        yc = yT.transpose(2, 3, 1, 0).reshape(TOK, C)
        y[core] = yc
    return y.reshape(B, T, C)


_CACHED = {}


def kernel(x, Wqkv, bqkv, Wout, bout):
    from concourse.bass_utils import run_bass_kernel_spmd
    if "nc" not in _CACHED:
        _CACHED["nc"] = _build(reps=1)
    in_maps = prep_inputs(x, Wqkv, bqkv, Wout, bout)
    res = run_bass_kernel_spmd(_CACHED["nc"], in_maps, list(range(N_CORES)))
    return assemble_output(res.results)
